# revision 9
# baseline (speedup 1.0000x reference)
"""Self-contained Trainium2 Bass kernel for a 12-head attention layer.

Problem: x[4,2048,768] -> attention(QKV projections, softmax, context),
NUM_HEADS=12, SIZE_PER_HEAD=64, additive mask from mask[4,2048].

Sharding over 8 NeuronCores: core c handles batch b=c//2 and head-group
hg=c%2 (6 heads, 384 feature columns).  Everything is local per core:
no collectives.

v2 design (ACT-bound): the per-core exp work (6 heads x 2048^2 = 25.2M
elements at 1 elem/lane/cycle @1.2GHz) floors the Scalar engine at
~220us, so the TensorEngine work is packed to fit UNDER that roof using
PE tile_position concurrency (measured: col-pairs 2.0x, denom quads
3.9x, row-pairs ~2x when stationaries are stable):

  per head-pair p (psum partitions stack head A at 0:63, B at 64:127):
    scores  S^T[t,f]: row-tiled pair (0,0)/(64,0), K=64 each, into two
            single-buffered psum tiles psA/psB [128,1024]
    exp     one ACT per head: Exp(psX + adder[ti]) -> bf16 SBUF tile
            (mask enters as per-partition ACT bias; zero for ones mask)
    ctx     col-tiled pair (0,0)/(0,64): K=128(t), M=64, accumulated
            over ti into ctx_ps[128=A|B, 1024]
    denom   ones-vector matmuls M=1, quad-packed at col strips
            0/32/64/96 into one psum bank, accumulated over ti
  normalization fully off the PE: DVE drain, DMA gather of denom rows,
  DVE reciprocal, gpsimd partition_broadcast, DVE multiply, DMA out.

Output per core: ctx^T [384,2048] f32; host transposes/concats.
"""

import numpy as np
import ml_dtypes

B, S, D = 4, 2048, 768
H, DH = 12, 64
HL = 6          # heads per core
DL = HL * DH    # 384 feature columns per core
NCORES = 8
P = 128
KO = 6          # full k-subtiles of the 768 contraction
NT = S // P     # 16 T-tiles
FB = 1024       # F block size
NFB = S // FB   # 2 F blocks

_CACHE = {}


def _build(with_bias=False, ncores=NCORES, dbg=False):
    import concourse.mybir as mybir
    import concourse.tile as tile
    from concourse import bacc

    dt = mybir.dt
    Exp = mybir.ActivationFunctionType.Exp
    Alu = mybir.AluOpType

    nc = bacc.Bacc("TRN2", target_bir_lowering=False, debug=False,
                   num_devices=ncores)

    DE = D + 1 if with_bias else D
    xT = nc.dram_tensor("xT", [DE, S], dt.bfloat16, kind="ExternalInput")
    wq = nc.dram_tensor("wq", [DE, DL], dt.bfloat16, kind="ExternalInput")
    wk = nc.dram_tensor("wk", [DE, DL], dt.bfloat16, kind="ExternalInput")
    wv = nc.dram_tensor("wv", [DE, DL], dt.bfloat16, kind="ExternalInput")
    adder = nc.dram_tensor("adder", [P, NT], dt.float32, kind="ExternalInput")
    out = nc.dram_tensor("out", [DL, S], dt.float32, kind="ExternalOutput")
    if dbg:
        d_cst = nc.dram_tensor("d_cst", [P, FB], dt.float32,
                               kind="ExternalOutput")
        d_dns = nc.dram_tensor("d_dns", [P, 512], dt.float32,
                               kind="ExternalOutput")
        d_rc = nc.dram_tensor("d_rc", [P, 16], dt.float32,
                              kind="ExternalOutput")
        d_bb = nc.dram_tensor("d_bb", [P, FB], dt.float32,
                              kind="ExternalOutput")
        d_exp = nc.dram_tensor("d_exp", [P, FB], dt.bfloat16,
                               kind="ExternalOutput")

    KE = KO + 1 if with_bias else KO

    with tile.TileContext(nc) as tc:
        with (
            tc.tile_pool(name="persist", bufs=1) as sb,
            tc.tile_pool(name="work", bufs=3) as work,
            tc.tile_pool(name="fin", bufs=2) as fin,
            tc.tile_pool(name="ps_s", bufs=1, space="PSUM") as ps_s,
            tc.tile_pool(name="ps_c", bufs=1, space="PSUM") as ps_c,
        ):
            # ---- input DMA ----
            xTs = sb.tile([P, KE, S], dt.bfloat16, tag="xTs")
            for ko in range(KO):
                nc.sync.dma_start(
                    xTs[:, ko, :], xT.ap()[ko * P:(ko + 1) * P, :])
            if with_bias:
                nc.sync.dma_start(xTs[0:1, KO, :], xT.ap()[D:D + 1, :])

            wqs = sb.tile([P, KE, DL], dt.bfloat16, tag="wqs")
            wks = sb.tile([P, KE, DL], dt.bfloat16, tag="wks")
            wvs = sb.tile([P, KE, DL], dt.bfloat16, tag="wvs")
            adder_sb = sb.tile([P, NT], dt.float32, tag="adder")
            ones_sb = sb.tile([P, 4], dt.bfloat16, tag="ones")
            nc.gpsimd.memset(ones_sb[:], 1.0)

            def dma_w(w_dram, w_sb):
                nc.sync.dma_start(
                    w_sb[:, 0:KO, :],
                    w_dram.ap()[0:D, :].rearrange("(ko p) m -> p ko m", p=P))
                if with_bias:
                    nc.sync.dma_start(w_sb[0:1, KO, :],
                                      w_dram.ap()[D:D + 1, :])

            dma_w(wq, wqs)
            dma_w(wk, wks)
            dma_w(wv, wvs)
            nc.sync.dma_start(adder_sb[:], adder.ap())

            # persistent projection outputs
            qt = sb.tile([P, 3, S], dt.bfloat16, tag="qt")   # Q^T/8 (+bias)
            kt = sb.tile([P, 3, S], dt.bfloat16, tag="kt")   # K^T (+bias)
            vp = sb.tile([P, NT, DL], dt.bfloat16, tag="vp")  # V' token-major

            # ---- projections ----
            def proj_qk(w_sb, dst, m, ns=(0, 1, 2, 3)):
                for n in ns:
                    pt = ps_c.tile([P, 512], dt.float32, tag="proj",
                                   name="pt")
                    for k in range(KE):
                        lhsT = (w_sb[:, k, m * P:(m + 1) * P] if k < KO
                                else w_sb[0:1, k, m * P:(m + 1) * P])
                        rhs = (xTs[:, k, n * 512:(n + 1) * 512] if k < KO
                               else xTs[0:1, k, n * 512:(n + 1) * 512])
                        nc.tensor.matmul(pt[:], lhsT, rhs,
                                         start=(k == 0), stop=(k == KE - 1))
                    nc.vector.tensor_copy(dst[:, m, n * 512:(n + 1) * 512],
                                          pt[:])

            def proj_v(mt):
                pt = ps_c.tile([P, 512], dt.float32, tag="proj", name="pt")
                for k in range(KE):
                    lhsT = (xTs[:, k, mt * P:(mt + 1) * P] if k < KO
                            else xTs[0:1, k, mt * P:(mt + 1) * P])
                    rhs = wvs[:, k, :] if k < KO else wvs[0:1, k, :]
                    nc.tensor.matmul(pt[:, :DL], lhsT, rhs,
                                     start=(k == 0), stop=(k == KE - 1))
                nc.vector.tensor_copy(vp[:, mt, :], pt[:, :DL])

            def epilogue(p, fb, ctx_ps, den_ps, last):
                # drain psum fast (DVE), then the off-PE normalize chain
                cst = fin.tile([P, FB], dt.float32, tag="cst", name="cst")
                nc.vector.tensor_copy(cst[:], ctx_ps[:])
                dns = fin.tile([P, 512], dt.float32, tag="dns", name="dns")
                nc.vector.tensor_copy(dns[:], den_ps[:])
                if dbg and p == 0 and fb == 0:
                    nc.sync.dma_start(d_cst.ap(), cst[:])
                    nc.sync.dma_start(d_dns.ap(), dns[:])
                # denom rows: p0=A-n0, p32=B-n0, p64=A-n1, p96=B-n1
                # gather to [128,16] so reciprocal uses all lanes
                dcol = fin.tile([P, 16], dt.float32, tag="dcol", name="dcol")
                for q in range(4):
                    nc.sync.dma_start(dcol[:, q * 4:(q + 1) * 4],
                                      dns[32 * q:32 * q + 1, :])
                rc = fin.tile([P, 16], dt.float32, tag="rc", name="rc")
                nc.vector.reciprocal(rc[:], dcol[:])
                # partition_broadcast always writes from partition 0, so
                # each head gets a full [128,FB] tile; slice with matching
                # base partitions in the multiplies.
                otc = fin.tile([P, FB], dt.float32, tag="otc",
                               bufs=3, name="otc")
                bbs = []
                for hip in range(2):
                    rrow = fin.tile([1, FB], dt.float32, tag="rrow",
                                    name="rrow")
                    for n in range(2):
                        q = hip + 2 * n
                        nc.sync.dma_start(rrow[:, n * 512:(n + 1) * 512],
                                          rc[:, q * 4:(q + 1) * 4])
                    bb = fin.tile([P, FB], dt.float32, tag=f"bb{hip}",
                                  name="bb")
                    nc.gpsimd.partition_broadcast(bb[:], rrow[:])
                    bbs.append(bb)
                    lo, hi = hip * DH, (hip + 1) * DH
                    nc.vector.tensor_tensor(otc[lo:hi, :], cst[lo:hi, :],
                                            bb[lo:hi, :], Alu.mult)
                if dbg and p == 0 and fb == 0:
                    nc.sync.dma_start(d_rc.ap(), rc[:])
                    nc.sync.dma_start(d_bb.ap()[0:DH, :], bbs[0][0:DH, :])
                    nc.sync.dma_start(d_bb.ap()[DH:P, :], bbs[1][DH:P, :])
                nc.sync.dma_start(
                    out.ap()[p * P:(p + 1) * P, fb * FB:(fb + 1) * FB],
                    otc[:])

            def attn_segment(p, fb, last=False):
                ctx_ps = ps_c.tile([P, FB], dt.float32, tag="ctx",
                                   name="ctx_ps")
                den_ps = ps_c.tile([P, 512], dt.float32, tag="den",
                                   name="den_ps")
                exp_tiles = {}

                def mm_ctx_den(ti):
                    etA, etB = exp_tiles.pop(ti)
                    first, fin_ = (ti == 0), (ti == NT - 1)
                    for n in range(2):
                        nc.tensor.matmul(
                            ctx_ps[0:DH, n * 512:(n + 1) * 512],
                            vp[:, ti, (2 * p) * DH:(2 * p + 1) * DH],
                            etA[:, n * 512:(n + 1) * 512],
                            start=first, stop=fin_)
                        nc.tensor.matmul(
                            ctx_ps[DH:P, n * 512:(n + 1) * 512],
                            vp[:, ti, (2 * p + 1) * DH:(2 * p + 2) * DH],
                            etB[:, n * 512:(n + 1) * 512],
                            start=first, stop=fin_)
                    # denom quad: q = hip + 2n at col strip 32q
                    for n in range(2):
                        for hip in range(2):
                            q = hip + 2 * n
                            et = etA if hip == 0 else etB
                            nc.tensor.matmul(
                                den_ps[32 * q:32 * q + 1, :],
                                ones_sb[:, q:q + 1],
                                et[:, n * 512:(n + 1) * 512],
                                start=first, stop=fin_,
                                tile_position=(0, 32 * q))

                for ti in range(NT):
                    if ti > 0:
                        mm_ctx_den(ti - 1)
                    psA = ps_s.tile([P, FB], dt.float32, tag="sa",
                                    name="psA")
                    psB = ps_s.tile([P, FB], dt.float32, tag="sb",
                                    name="psB")
                    for n in range(2):
                        fcol = fb * FB + n * 512
                        nc.tensor.matmul(
                            psA[:, n * 512:(n + 1) * 512],
                            kt[0:DH, p, ti * P:(ti + 1) * P],
                            qt[0:DH, p, fcol:fcol + 512],
                            start=True, stop=True)
                    for n in range(2):
                        fcol = fb * FB + n * 512
                        nc.tensor.matmul(
                            psB[:, n * 512:(n + 1) * 512],
                            kt[DH:P, p, ti * P:(ti + 1) * P],
                            qt[DH:P, p, fcol:fcol + 512],
                            start=True, stop=True)
                    etA = work.tile([P, FB], dt.bfloat16, tag="expA",
                                    name="etA")
                    nc.scalar.activation(etA[:], psA[:], Exp,
                                         bias=adder_sb[:, ti:ti + 1],
                                         scale=1.0)
                    etB = work.tile([P, FB], dt.bfloat16, tag="expB",
                                    name="etB")
                    nc.scalar.activation(etB[:], psB[:], Exp,
                                         bias=adder_sb[:, ti:ti + 1],
                                         scale=1.0)
                    if dbg and p == 0 and fb == 0 and ti == 0:
                        nc.sync.dma_start(d_exp.ap(), etA[:])
                    exp_tiles[ti] = (etA, etB)
                mm_ctx_den(NT - 1)
                epilogue(p, fb, ctx_ps, den_ps, last)

            # PE warm-up: garbage matmuls with no input deps run during the
            # initial DMA wait, releasing the HAM clock throttle.
            warm = sb.tile([P, 512], dt.bfloat16, tag="warm")
            nc.gpsimd.memset(warm[:], 0.0)
            wexp = sb.tile([P, 1], dt.bfloat16, tag="wexp")
            nc.scalar.activation(wexp[:], warm[:, 0:1], Exp)
            wpt = ps_s.tile([P, 512], dt.float32, tag="sa", name="wpt")
            for wi in range(20):
                nc.tensor.matmul(wpt[:], warm[:, 0:P], warm[:],
                                 start=(wi == 0), stop=(wi == 19))
            wpt2 = ps_s.tile([P, 512], dt.float32, tag="sb", name="wpt2")
            for wi in range(28):
                nc.tensor.matmul(wpt2[:, 0:256], warm[:, 0:P],
                                 warm[:, 0:256],
                                 start=(wi == 0), stop=(wi == 27))

            proj_qk(wqs, qt, 0, ns=(0, 1))
            proj_qk(wks, kt, 0)
            proj_qk(wqs, qt, 0, ns=(2, 3))
            for mt in range(NT):
                proj_v(mt)
            with tc.high_priority():
                attn_segment(0, 0)
                attn_segment(0, 1)
            proj_qk(wqs, qt, 1)
            proj_qk(wks, kt, 1)
            with tc.high_priority():
                attn_segment(1, 0)
                attn_segment(1, 1)
            proj_qk(wqs, qt, 2)
            proj_qk(wks, kt, 2)
            with tc.high_priority():
                attn_segment(2, 0)
                attn_segment(2, 1, last=True)

    nc.compile()
    return nc


def _prep_core_inputs(c, x, Wq, bq, Wk, bk, Wv, bv, mask, with_bias):
    bf16 = ml_dtypes.bfloat16
    b, hg = c // 2, c % 2
    cols = slice(hg * DL, (hg + 1) * DL)
    DE = D + 1 if with_bias else D

    xT_aug = np.empty((DE, S), dtype=bf16)
    xT_aug[:D] = x[b].T.astype(bf16)
    if with_bias:
        xT_aug[D] = np.float32(1.0)

    wq_aug = np.empty((DE, DL), dtype=bf16)
    wq_aug[:D] = (Wq[:, cols] / 8.0).astype(bf16)
    wk_aug = np.empty((DE, DL), dtype=bf16)
    wk_aug[:D] = Wk[:, cols].astype(bf16)
    wv_aug = np.empty((DE, DL), dtype=bf16)
    wv_aug[:D] = Wv[:, cols].astype(bf16)
    if with_bias:
        wq_aug[D] = (bq[cols] / 8.0).astype(bf16)
        wk_aug[D] = bk[cols].astype(bf16)
        wv_aug[D] = bv[cols].astype(bf16)

    add = ((mask[b].astype(np.float32) - 1.0) * 10000.0)
    adder_t = add.reshape(NT, P).T.copy()   # [128,16]: [p, ti]

    return {"xT": xT_aug, "wq": wq_aug, "wk": wk_aug, "wv": wv_aug,
            "adder": np.ascontiguousarray(adder_t, dtype=np.float32)}


def kernel(x, Wq, bq, Wk, bk, Wv, bv, mask, _trace=False):
    from concourse.bass_utils import run_bass_kernel_spmd

    x = np.asarray(x, dtype=np.float32)
    Wq = np.asarray(Wq, dtype=np.float32)
    bq = np.asarray(bq, dtype=np.float32)
    Wk = np.asarray(Wk, dtype=np.float32)
    bk = np.asarray(bk, dtype=np.float32)
    Wv = np.asarray(Wv, dtype=np.float32)
    bv = np.asarray(bv, dtype=np.float32)
    mask = np.asarray(mask)

    with_bias = bool(bq.any() or bk.any() or bv.any())
    key = ("nc", with_bias)
    if key not in _CACHE:
        _CACHE[key] = _build(with_bias=with_bias)
    nc = _CACHE[key]

    in_maps = [_prep_core_inputs(c, x, Wq, bq, Wk, bk, Wv, bv, mask,
                                 with_bias)
               for c in range(NCORES)]
    res = run_bass_kernel_spmd(nc, in_maps, core_ids=list(range(NCORES)),
                               trace=_trace)
    if _trace:
        _CACHE["last_result"] = res

    full = np.empty((B, S, D), dtype=np.float32)
    for c in range(NCORES):
        b, hg = c // 2, c % 2
        full[b, :, hg * DL:(hg + 1) * DL] = res.results[c]["out"].T
    return full


# revision 12
# speedup vs baseline: 1.0566x; 1.0566x over previous
"""Self-contained Trainium2 Bass kernel for a 12-head attention layer.

Problem: x[4,2048,768] -> attention(QKV projections, softmax, context),
NUM_HEADS=12, SIZE_PER_HEAD=64, additive mask from mask[4,2048].

Sharding over 8 NeuronCores: core c handles batch b=c//2 and head-group
hg=c%2 (6 heads, 384 feature columns).  Everything is local per core:
no collectives.

v3 design (ACT-bound): the per-core exp work (6 heads x 2048^2 = 25.2M
elements at 1 elem/lane/cycle @1.2GHz + 352c/inst) floors the Scalar
engine at ~220us with N=1024 ACTs, so the TensorEngine stream is
organized to fit just under that roof:

  per head-pair p, f-chunk g (512 wide), t-tile ti:
    scores  S^T[t,f]: head A -> psS[:,0:512], head B -> psS[:,512:1024]
            (one [128,1024] psum tile, two banks, double-buffered)
    exp     ONE ACT N=1024 over both heads' chunks:
            Exp(psS + adder[ti]) -> bf16 [128,1024]  (mask = ACT bias)
    ctx     per head: [65,512] psum accumulated over ti; the V tiles
            carry a 65th ones-column so row 64 is the softmax
            denominator (no separate denominator matmuls)
  normalization fully off the PE: DVE drain, DMA gather of denom rows,
  DVE reciprocal, gpsimd partition_broadcast, DVE multiply, DMA out.

Per-cycle budget: ACT 1147ns vs PE 2x scores + 2x ctx ~ 950ns + shared
projection work ~ 290ns -> both engines ~saturated, ACT binding.

Output per core: ctx^T [384,2048] f32; host transposes/concats.
"""

import numpy as np
import ml_dtypes

B, S, D = 4, 2048, 768
H, DH = 12, 64
HL = 6          # heads per core
DL = HL * DH    # 384 feature columns per core
NCORES = 8
P = 128
KO = 6          # full k-subtiles of the 768 contraction
NT = S // P     # 16 T-tiles
NG = 4          # f-chunks of 512 per head

_CACHE = {}


def _build(with_bias=False, ncores=NCORES):
    import concourse.mybir as mybir
    import concourse.tile as tile
    from concourse import bacc

    dt = mybir.dt
    Exp = mybir.ActivationFunctionType.Exp
    Alu = mybir.AluOpType

    nc = bacc.Bacc("TRN2", target_bir_lowering=False, debug=False,
                   num_devices=ncores)

    DE = D + 1 if with_bias else D
    WVC = HL * (DH + 1) if with_bias else DL   # 390 vs 384
    xT = nc.dram_tensor("xT", [DE, S], dt.bfloat16, kind="ExternalInput")
    wq = nc.dram_tensor("wq", [DE, DL], dt.bfloat16, kind="ExternalInput")
    wk = nc.dram_tensor("wk", [DE, DL], dt.bfloat16, kind="ExternalInput")
    wv = nc.dram_tensor("wv", [DE, WVC], dt.bfloat16, kind="ExternalInput")
    adder = nc.dram_tensor("adder", [P, NT], dt.float32, kind="ExternalInput")
    out = nc.dram_tensor("out", [DL, S], dt.float32, kind="ExternalOutput")

    KE = KO + 1 if with_bias else KO

    with tile.TileContext(nc) as tc:
        with (
            tc.tile_pool(name="persist", bufs=1) as sb,
            tc.tile_pool(name="work", bufs=3) as work,
            tc.tile_pool(name="fin", bufs=2) as fin,
            tc.tile_pool(name="ps_s", bufs=2, space="PSUM") as ps_s,
            tc.tile_pool(name="ps_c", bufs=1, space="PSUM") as ps_c,
        ):
            # ---- input DMA ----
            xTs = sb.tile([P, KE, S], dt.bfloat16, tag="xTs")
            for ko in range(KO):
                nc.sync.dma_start(
                    xTs[:, ko, :], xT.ap()[ko * P:(ko + 1) * P, :])
            if with_bias:
                nc.sync.dma_start(xTs[0:1, KO, :], xT.ap()[D:D + 1, :])

            wqs = sb.tile([P, KE, DL], dt.bfloat16, tag="wqs")
            wks = sb.tile([P, KE, DL], dt.bfloat16, tag="wks")
            wvs = sb.tile([P, KE, WVC], dt.bfloat16, tag="wvs")
            adder_sb = sb.tile([P, NT], dt.float32, tag="adder")

            def dma_w(w_dram, w_sb, cols):
                nc.sync.dma_start(
                    w_sb[:, 0:KO, 0:cols],
                    w_dram.ap()[0:D, :].rearrange("(ko p) m -> p ko m", p=P))
                if with_bias:
                    nc.sync.dma_start(w_sb[0:1, KO, 0:cols],
                                      w_dram.ap()[D:D + 1, :])

            dma_w(wq, wqs, DL)
            dma_w(wk, wks, DL)
            dma_w(wv, wvs, WVC)
            nc.sync.dma_start(adder_sb[:], adder.ap())

            # persistent projection outputs
            qt = sb.tile([P, 3, S], dt.bfloat16, tag="qt")   # Q^T/8 (+bias)
            kt = sb.tile([P, 3, S], dt.bfloat16, tag="kt")   # K^T (+bias)
            # V' token-major, 65-col head blocks (65th col = ones -> denom)
            vp = sb.tile([P, NT, HL, DH + 1], dt.bfloat16, tag="vp")
            if not with_bias:
                nc.gpsimd.memset(vp[:, :, :, DH:DH + 1], 1.0)

            # ---- projections ----
            def proj_qk(w_sb, dst, m, ns=(0, 1, 2, 3)):
                for n in ns:
                    pt = ps_c.tile([P, 512], dt.float32, tag="proj",
                                   name="pt")
                    for k in range(KE):
                        lhsT = (w_sb[:, k, m * P:(m + 1) * P] if k < KO
                                else w_sb[0:1, k, m * P:(m + 1) * P])
                        rhs = (xTs[:, k, n * 512:(n + 1) * 512] if k < KO
                               else xTs[0:1, k, n * 512:(n + 1) * 512])
                        nc.tensor.matmul(pt[:], lhsT, rhs,
                                         start=(k == 0), stop=(k == KE - 1))
                    nc.vector.tensor_copy(dst[:, m, n * 512:(n + 1) * 512],
                                          pt[:])

            def proj_v(mt):
                pt = ps_c.tile([P, 512], dt.float32, tag="proj", name="pt")
                for k in range(KE):
                    lhsT = (xTs[:, k, mt * P:(mt + 1) * P] if k < KO
                            else xTs[0:1, k, mt * P:(mt + 1) * P])
                    rhs = wvs[:, k, 0:WVC] if k < KO else wvs[0:1, k, 0:WVC]
                    nc.tensor.matmul(pt[:, :WVC], lhsT, rhs,
                                     start=(k == 0), stop=(k == KE - 1))
                if with_bias:
                    nc.vector.tensor_copy(
                        vp[:, mt, :, :],
                        pt[:, :WVC].rearrange("p (h c) -> p h c", h=HL))
                else:
                    nc.vector.tensor_copy(
                        vp[:, mt, :, 0:DH],
                        pt[:, :DL].rearrange("p (h c) -> p h c", h=HL))

            def epilogue(p, g, ctx_ps):
                # ctx_ps: [hip] -> [DH+1, 512] psum; row DH = denominator.
                # Drain psum fast (DVE), then the off-PE normalize chain.
                # DVE lanes are partition-locked, so each head stages at
                # base partition 0 and the output DMA moves the rows.
                csts = []
                for hip in range(2):
                    cst = fin.tile([DH + 1, 512], dt.float32,
                                   tag=f"cst{hip}", name="cst", bufs=2)
                    nc.vector.tensor_copy(cst[:], ctx_ps[hip][:])
                    csts.append(cst)
                dcol = fin.tile([P, 8], dt.float32, tag="dcol", name="dcol")
                for hip in range(2):
                    nc.sync.dma_start(dcol[:, hip * 4:(hip + 1) * 4],
                                      csts[hip][DH:DH + 1, :])
                rc = fin.tile([P, 8], dt.float32, tag="rc", name="rc")
                nc.vector.reciprocal(rc[:], dcol[:])
                for hip in range(2):
                    rrow = fin.tile([1, 512], dt.float32, tag="rrow",
                                    name="rrow")
                    nc.sync.dma_start(rrow[:],
                                      rc[:, hip * 4:(hip + 1) * 4])
                    bb = fin.tile([DH, 512], dt.float32, tag="bb",
                                  name="bb")
                    nc.gpsimd.partition_broadcast(bb[:], rrow[:])
                    otc = fin.tile([DH, 512], dt.float32, tag="otc",
                                   bufs=3, name="otc")
                    nc.vector.tensor_tensor(otc[:], csts[hip][0:DH, :],
                                            bb[:], Alu.mult)
                    nc.sync.dma_start(
                        out.ap()[p * P + hip * DH:p * P + (hip + 1) * DH,
                                 g * 512:(g + 1) * 512],
                        otc[:])

            def attn_segment(p, g):
                ctx_ps = [
                    ps_c.tile([DH + 1, 512], dt.float32, tag=f"c{hip}",
                              name="ctx_ps")
                    for hip in range(2)
                ]
                exp_tiles = {}

                def mm_ctx(ti):
                    et = exp_tiles.pop(ti)
                    first, fin_ = (ti == 0), (ti == NT - 1)
                    for hip in range(2):
                        nc.tensor.matmul(
                            ctx_ps[hip][:],
                            vp[:, ti, 2 * p + hip, :],
                            et[:, hip * 512:(hip + 1) * 512],
                            start=first, stop=fin_)

                for ti in range(NT):
                    if ti > 0:
                        mm_ctx(ti - 1)
                    psS = ps_s.tile([P, 1024], dt.float32, tag="s",
                                    name="psS")
                    nc.tensor.matmul(
                        psS[:, 0:512],
                        kt[0:DH, p, ti * P:(ti + 1) * P],
                        qt[0:DH, p, g * 512:(g + 1) * 512],
                        start=True, stop=True)
                    nc.tensor.matmul(
                        psS[:, 512:1024],
                        kt[DH:P, p, ti * P:(ti + 1) * P],
                        qt[DH:P, p, g * 512:(g + 1) * 512],
                        start=True, stop=True)
                    et = work.tile([P, 1024], dt.bfloat16, tag="exp",
                                   name="et")
                    nc.scalar.activation(et[:], psS[:], Exp,
                                         bias=adder_sb[:, ti:ti + 1],
                                         scale=1.0)
                    exp_tiles[ti] = et
                mm_ctx(NT - 1)
                epilogue(p, g, ctx_ps)

            # PE warm-up: garbage matmuls with no input deps run during the
            # initial DMA wait, releasing the HAM clock throttle.
            warm = sb.tile([P, 512], dt.bfloat16, tag="warm")
            nc.gpsimd.memset(warm[:], 0.0)
            wexp = sb.tile([P, 1], dt.bfloat16, tag="wexp")
            nc.scalar.activation(wexp[:], warm[:, 0:1], Exp)
            wpt = ps_s.tile([P, 1024], dt.float32, tag="s", name="wpt")
            for wi in range(20):
                nc.tensor.matmul(wpt[:, 0:512], warm[:, 0:P], warm[:],
                                 start=(wi == 0), stop=(wi == 19))
            wpt2 = ps_s.tile([P, 1024], dt.float32, tag="s", name="wpt2")
            for wi in range(28):
                nc.tensor.matmul(wpt2[:, 0:256], warm[:, 0:P],
                                 warm[:, 0:256],
                                 start=(wi == 0), stop=(wi == 27))

            proj_qk(wqs, qt, 0, ns=(0, 1))
            proj_qk(wks, kt, 0)
            proj_qk(wqs, qt, 0, ns=(2, 3))
            for mt in range(NT):
                proj_v(mt)
            with tc.high_priority():
                for g in range(NG):
                    attn_segment(0, g)
            proj_qk(wqs, qt, 1)
            proj_qk(wks, kt, 1)
            with tc.high_priority():
                for g in range(NG):
                    attn_segment(1, g)
            proj_qk(wqs, qt, 2)
            proj_qk(wks, kt, 2)
            with tc.high_priority():
                for g in range(NG):
                    attn_segment(2, g)

    nc.compile()
    return nc


def _prep_core_inputs(c, x, Wq, bq, Wk, bk, Wv, bv, mask, with_bias):
    bf16 = ml_dtypes.bfloat16
    b, hg = c // 2, c % 2
    cols = slice(hg * DL, (hg + 1) * DL)
    DE = D + 1 if with_bias else D

    xT_aug = np.empty((DE, S), dtype=bf16)
    xT_aug[:D] = x[b].T.astype(bf16)
    if with_bias:
        xT_aug[D] = np.float32(1.0)

    wq_aug = np.empty((DE, DL), dtype=bf16)
    wq_aug[:D] = (Wq[:, cols] / 8.0).astype(bf16)
    wk_aug = np.empty((DE, DL), dtype=bf16)
    wk_aug[:D] = Wk[:, cols].astype(bf16)
    if with_bias:
        wq_aug[D] = (bq[cols] / 8.0).astype(bf16)
        wk_aug[D] = bk[cols].astype(bf16)
        wv_aug = np.zeros((DE, HL * (DH + 1)), dtype=bf16)
        wv_loc = Wv[:, cols].astype(np.float32)
        bv_loc = bv[cols].astype(np.float32)
        for j in range(HL):
            wv_aug[:D, j * (DH + 1):j * (DH + 1) + DH] = \
                wv_loc[:, j * DH:(j + 1) * DH].astype(bf16)
            wv_aug[D, j * (DH + 1):j * (DH + 1) + DH] = \
                bv_loc[j * DH:(j + 1) * DH].astype(bf16)
            wv_aug[D, j * (DH + 1) + DH] = np.float32(1.0)
    else:
        wv_aug = np.empty((DE, DL), dtype=bf16)
        wv_aug[:D] = Wv[:, cols].astype(bf16)

    add = ((mask[b].astype(np.float32) - 1.0) * 10000.0)
    adder_t = add.reshape(NT, P).T.copy()   # [128,16]: [p, ti]

    return {"xT": xT_aug, "wq": wq_aug, "wk": wk_aug, "wv": wv_aug,
            "adder": np.ascontiguousarray(adder_t, dtype=np.float32)}


def kernel(x, Wq, bq, Wk, bk, Wv, bv, mask, _trace=False):
    from concourse.bass_utils import run_bass_kernel_spmd

    x = np.asarray(x, dtype=np.float32)
    Wq = np.asarray(Wq, dtype=np.float32)
    bq = np.asarray(bq, dtype=np.float32)
    Wk = np.asarray(Wk, dtype=np.float32)
    bk = np.asarray(bk, dtype=np.float32)
    Wv = np.asarray(Wv, dtype=np.float32)
    bv = np.asarray(bv, dtype=np.float32)
    mask = np.asarray(mask)

    with_bias = bool(bq.any() or bk.any() or bv.any())
    key = ("nc", with_bias)
    if key not in _CACHE:
        _CACHE[key] = _build(with_bias=with_bias)
    nc = _CACHE[key]

    in_maps = [_prep_core_inputs(c, x, Wq, bq, Wk, bk, Wv, bv, mask,
                                 with_bias)
               for c in range(NCORES)]
    res = run_bass_kernel_spmd(nc, in_maps, core_ids=list(range(NCORES)),
                               trace=_trace)
    if _trace:
        _CACHE["last_result"] = res

    full = np.empty((B, S, D), dtype=np.float32)
    for c in range(NCORES):
        b, hg = c // 2, c % 2
        full[b, :, hg * DL:(hg + 1) * DL] = res.results[c]["out"].T
    return full


# revision 13
# speedup vs baseline: 1.2370x; 1.1707x over previous
"""Self-contained Trainium2 Bass kernel for a 12-head attention layer.

Problem: x[4,2048,768] -> attention(QKV projections, softmax, context),
NUM_HEADS=12, SIZE_PER_HEAD=64, additive mask from mask[4,2048].

Sharding over 8 NeuronCores: core c handles batch b=c//2 and head-group
hg=c%2 (6 heads, 384 feature columns).  Everything is local per core:
no collectives.

v3 design (ACT-bound): the per-core exp work (6 heads x 2048^2 = 25.2M
elements at 1 elem/lane/cycle @1.2GHz + 352c/inst) floors the Scalar
engine at ~220us with N=1024 ACTs, so the TensorEngine stream is
organized to fit just under that roof:

  per head-pair p, f-chunk g (512 wide), t-tile ti:
    scores  S^T[t,f]: head A -> psS[:,0:512], head B -> psS[:,512:1024]
            (one [128,1024] psum tile, two banks, double-buffered)
    exp     ONE ACT N=1024 over both heads' chunks:
            Exp(psS + adder[ti]) -> bf16 [128,1024]  (mask = ACT bias)
    ctx     per head: [65,512] psum accumulated over ti; the V tiles
            carry a 65th ones-column so row 64 is the softmax
            denominator (no separate denominator matmuls)
  normalization fully off the PE: DVE drain, DMA gather of denom rows,
  DVE reciprocal, gpsimd partition_broadcast, DVE multiply, DMA out.

Per-cycle budget: ACT 1147ns vs PE 2x scores + 2x ctx ~ 950ns + shared
projection work ~ 290ns -> both engines ~saturated, ACT binding.

Output per core: ctx^T [384,2048] f32; host transposes/concats.
"""

import numpy as np
import ml_dtypes

B, S, D = 4, 2048, 768
H, DH = 12, 64
HL = 6          # heads per core
DL = HL * DH    # 384 feature columns per core
NCORES = 8
P = 128
KO = 6          # full k-subtiles of the 768 contraction
NT = S // P     # 16 T-tiles
NG = 4          # f-chunks of 512 per head

_CACHE = {}


def _build(with_bias=False, ncores=NCORES):
    import concourse.mybir as mybir
    import concourse.tile as tile
    from concourse import bacc

    dt = mybir.dt
    Exp = mybir.ActivationFunctionType.Exp
    Alu = mybir.AluOpType

    nc = bacc.Bacc("TRN2", target_bir_lowering=False, debug=False,
                   num_devices=ncores)

    DE = D + 1 if with_bias else D
    WVC = HL * (DH + 1) if with_bias else DL   # 390 vs 384
    xT = nc.dram_tensor("xT", [DE, S], dt.bfloat16, kind="ExternalInput")
    wq = nc.dram_tensor("wq", [DE, DL], dt.bfloat16, kind="ExternalInput")
    wk = nc.dram_tensor("wk", [DE, DL], dt.bfloat16, kind="ExternalInput")
    wv = nc.dram_tensor("wv", [DE, WVC], dt.bfloat16, kind="ExternalInput")
    adder = nc.dram_tensor("adder", [P, NT], dt.float32, kind="ExternalInput")
    out = nc.dram_tensor("out", [DL, S], dt.float32, kind="ExternalOutput")

    KE = KO + 1 if with_bias else KO

    with tile.TileContext(nc) as tc:
        with (
            tc.tile_pool(name="persist", bufs=1) as sb,
            tc.tile_pool(name="work", bufs=3) as work,
            tc.tile_pool(name="fin", bufs=2) as fin,
            tc.tile_pool(name="ps_s", bufs=2, space="PSUM") as ps_s,
            tc.tile_pool(name="ps_c", bufs=1, space="PSUM") as ps_c,
        ):
            # ---- input DMA ----
            xTs = sb.tile([P, KE, S], dt.bfloat16, tag="xTs")
            for ko in range(KO):
                nc.sync.dma_start(
                    xTs[:, ko, :], xT.ap()[ko * P:(ko + 1) * P, :])
            if with_bias:
                nc.sync.dma_start(xTs[0:1, KO, :], xT.ap()[D:D + 1, :])

            wqs = sb.tile([P, KE, DL], dt.bfloat16, tag="wqs")
            wks = sb.tile([P, KE, DL], dt.bfloat16, tag="wks")
            wvs = sb.tile([P, KE, WVC], dt.bfloat16, tag="wvs")
            adder_sb = sb.tile([P, NT], dt.float32, tag="adder")

            def dma_w(w_dram, w_sb, cols):
                nc.sync.dma_start(
                    w_sb[:, 0:KO, 0:cols],
                    w_dram.ap()[0:D, :].rearrange("(ko p) m -> p ko m", p=P))
                if with_bias:
                    nc.sync.dma_start(w_sb[0:1, KO, 0:cols],
                                      w_dram.ap()[D:D + 1, :])

            dma_w(wq, wqs, DL)
            dma_w(wk, wks, DL)
            dma_w(wv, wvs, WVC)
            nc.sync.dma_start(adder_sb[:], adder.ap())

            # persistent projection outputs
            qt = sb.tile([P, 3, S], dt.bfloat16, tag="qt")   # Q^T/8 (+bias)
            kt = sb.tile([P, 3, S], dt.bfloat16, tag="kt")   # K^T (+bias)
            # V' token-major, 65-col head blocks (65th col = ones -> denom)
            vp = sb.tile([P, NT, HL, DH + 1], dt.bfloat16, tag="vp")
            if not with_bias:
                nc.gpsimd.memset(vp[:, :, :, DH:DH + 1], 1.0)

            # ---- projections ----
            def proj_qk(w_sb, dst, m, ns=(0, 1, 2, 3)):
                for n in ns:
                    pt = ps_c.tile([P, 512], dt.float32, tag="proj",
                                   name="pt")
                    for k in range(KE):
                        lhsT = (w_sb[:, k, m * P:(m + 1) * P] if k < KO
                                else w_sb[0:1, k, m * P:(m + 1) * P])
                        rhs = (xTs[:, k, n * 512:(n + 1) * 512] if k < KO
                               else xTs[0:1, k, n * 512:(n + 1) * 512])
                        nc.tensor.matmul(pt[:], lhsT, rhs,
                                         start=(k == 0), stop=(k == KE - 1))
                    nc.vector.tensor_copy(dst[:, m, n * 512:(n + 1) * 512],
                                          pt[:])

            def proj_v(mt):
                pt = ps_c.tile([P, 512], dt.float32, tag="proj", name="pt")
                for k in range(KE):
                    lhsT = (xTs[:, k, mt * P:(mt + 1) * P] if k < KO
                            else xTs[0:1, k, mt * P:(mt + 1) * P])
                    rhs = wvs[:, k, 0:WVC] if k < KO else wvs[0:1, k, 0:WVC]
                    nc.tensor.matmul(pt[:, :WVC], lhsT, rhs,
                                     start=(k == 0), stop=(k == KE - 1))
                if with_bias:
                    nc.vector.tensor_copy(
                        vp[:, mt, :, :],
                        pt[:, :WVC].rearrange("p (h c) -> p h c", h=HL))
                else:
                    nc.vector.tensor_copy(
                        vp[:, mt, :, 0:DH],
                        pt[:, :DL].rearrange("p (h c) -> p h c", h=HL))

            def epilogue(p, g, ctx_ps):
                # ctx_ps: [hip] -> [DH+1, 512] psum; row DH = denominator.
                # Drain psum fast (DVE), then the off-PE normalize chain.
                # DVE lanes are partition-locked, so each head stages at
                # base partition 0 and the output DMA moves the rows.
                csts = []
                for hip in range(2):
                    cst = fin.tile([DH + 1, 512], dt.float32,
                                   tag=f"cst{hip}", name="cst", bufs=2)
                    nc.vector.tensor_copy(cst[:], ctx_ps[hip][:])
                    csts.append(cst)
                dcol = fin.tile([P, 8], dt.float32, tag="dcol", name="dcol")
                for hip in range(2):
                    nc.sync.dma_start(dcol[:, hip * 4:(hip + 1) * 4],
                                      csts[hip][DH:DH + 1, :])
                rc = fin.tile([P, 8], dt.float32, tag="rc", name="rc")
                nc.vector.reciprocal(rc[:], dcol[:])
                for hip in range(2):
                    rrow = fin.tile([1, 512], dt.float32, tag="rrow",
                                    name="rrow")
                    nc.sync.dma_start(rrow[:],
                                      rc[:, hip * 4:(hip + 1) * 4])
                    bb = fin.tile([DH, 512], dt.float32, tag="bb",
                                  name="bb")
                    nc.gpsimd.partition_broadcast(bb[:], rrow[:])
                    otc = fin.tile([DH, 512], dt.float32, tag="otc",
                                   bufs=3, name="otc")
                    nc.vector.tensor_tensor(otc[:], csts[hip][0:DH, :],
                                            bb[:], Alu.mult)
                    nc.sync.dma_start(
                        out.ap()[p * P + hip * DH:p * P + (hip + 1) * DH,
                                 g * 512:(g + 1) * 512],
                        otc[:])

            def attn_segment(p, g):
                ctx_ps = [
                    ps_c.tile([DH + 1, 512], dt.float32, tag=f"c{hip}",
                              name="ctx_ps")
                    for hip in range(2)
                ]
                exp_tiles = {}

                def mm_ctx(ti):
                    et = exp_tiles.pop(ti)
                    first, fin_ = (ti == 0), (ti == NT - 1)
                    for hip in range(2):
                        nc.tensor.matmul(
                            ctx_ps[hip][:],
                            vp[:, ti, 2 * p + hip, :],
                            et[:, hip * 512:(hip + 1) * 512],
                            start=first, stop=fin_)

                for ti in range(NT):
                    # scores(ti) first: their deps (psS slot) are ready
                    # early, so the PE never heads-of-line-blocks on the
                    # previous exp; ctx(ti-1) follows and overlaps exp(ti).
                    psS = ps_s.tile([P, 1024], dt.float32, tag="s",
                                    name="psS")
                    nc.tensor.matmul(
                        psS[:, 0:512],
                        kt[0:DH, p, ti * P:(ti + 1) * P],
                        qt[0:DH, p, g * 512:(g + 1) * 512],
                        start=True, stop=True)
                    nc.tensor.matmul(
                        psS[:, 512:1024],
                        kt[DH:P, p, ti * P:(ti + 1) * P],
                        qt[DH:P, p, g * 512:(g + 1) * 512],
                        start=True, stop=True)
                    et = work.tile([P, 1024], dt.bfloat16, tag="exp",
                                   name="et")
                    nc.scalar.activation(et[:], psS[:], Exp,
                                         bias=adder_sb[:, ti:ti + 1],
                                         scale=1.0)
                    exp_tiles[ti] = et
                    if ti > 0:
                        mm_ctx(ti - 1)
                mm_ctx(NT - 1)
                epilogue(p, g, ctx_ps)

            # PE warm-up: garbage matmuls with no input deps run during the
            # initial DMA wait, releasing the HAM clock throttle.
            warm = sb.tile([P, 512], dt.bfloat16, tag="warm")
            nc.gpsimd.memset(warm[:], 0.0)
            wexp = sb.tile([P, 1], dt.bfloat16, tag="wexp")
            nc.scalar.activation(wexp[:], warm[:, 0:1], Exp)
            wpt = ps_s.tile([P, 1024], dt.float32, tag="s", name="wpt")
            for wi in range(20):
                nc.tensor.matmul(wpt[:, 0:512], warm[:, 0:P], warm[:],
                                 start=(wi == 0), stop=(wi == 19))
            wpt2 = ps_s.tile([P, 1024], dt.float32, tag="s", name="wpt2")
            for wi in range(28):
                nc.tensor.matmul(wpt2[:, 0:256], warm[:, 0:P],
                                 warm[:, 0:256],
                                 start=(wi == 0), stop=(wi == 27))

            proj_qk(wqs, qt, 0, ns=(0, 1))
            proj_qk(wks, kt, 0)
            proj_qk(wqs, qt, 0, ns=(2, 3))
            for mt in range(NT):
                proj_v(mt)
            with tc.high_priority():
                for g in range(NG):
                    attn_segment(0, g)
            proj_qk(wqs, qt, 1)
            proj_qk(wks, kt, 1)
            with tc.high_priority():
                for g in range(NG):
                    attn_segment(1, g)
            proj_qk(wqs, qt, 2)
            proj_qk(wks, kt, 2)
            with tc.high_priority():
                for g in range(NG):
                    attn_segment(2, g)

    nc.compile()
    return nc


def _prep_core_inputs(c, x, Wq, bq, Wk, bk, Wv, bv, mask, with_bias):
    bf16 = ml_dtypes.bfloat16
    b, hg = c // 2, c % 2
    cols = slice(hg * DL, (hg + 1) * DL)
    DE = D + 1 if with_bias else D

    xT_aug = np.empty((DE, S), dtype=bf16)
    xT_aug[:D] = x[b].T.astype(bf16)
    if with_bias:
        xT_aug[D] = np.float32(1.0)

    wq_aug = np.empty((DE, DL), dtype=bf16)
    wq_aug[:D] = (Wq[:, cols] / 8.0).astype(bf16)
    wk_aug = np.empty((DE, DL), dtype=bf16)
    wk_aug[:D] = Wk[:, cols].astype(bf16)
    if with_bias:
        wq_aug[D] = (bq[cols] / 8.0).astype(bf16)
        wk_aug[D] = bk[cols].astype(bf16)
        wv_aug = np.zeros((DE, HL * (DH + 1)), dtype=bf16)
        wv_loc = Wv[:, cols].astype(np.float32)
        bv_loc = bv[cols].astype(np.float32)
        for j in range(HL):
            wv_aug[:D, j * (DH + 1):j * (DH + 1) + DH] = \
                wv_loc[:, j * DH:(j + 1) * DH].astype(bf16)
            wv_aug[D, j * (DH + 1):j * (DH + 1) + DH] = \
                bv_loc[j * DH:(j + 1) * DH].astype(bf16)
            wv_aug[D, j * (DH + 1) + DH] = np.float32(1.0)
    else:
        wv_aug = np.empty((DE, DL), dtype=bf16)
        wv_aug[:D] = Wv[:, cols].astype(bf16)

    add = ((mask[b].astype(np.float32) - 1.0) * 10000.0)
    adder_t = add.reshape(NT, P).T.copy()   # [128,16]: [p, ti]

    return {"xT": xT_aug, "wq": wq_aug, "wk": wk_aug, "wv": wv_aug,
            "adder": np.ascontiguousarray(adder_t, dtype=np.float32)}


def kernel(x, Wq, bq, Wk, bk, Wv, bv, mask, _trace=False):
    from concourse.bass_utils import run_bass_kernel_spmd

    x = np.asarray(x, dtype=np.float32)
    Wq = np.asarray(Wq, dtype=np.float32)
    bq = np.asarray(bq, dtype=np.float32)
    Wk = np.asarray(Wk, dtype=np.float32)
    bk = np.asarray(bk, dtype=np.float32)
    Wv = np.asarray(Wv, dtype=np.float32)
    bv = np.asarray(bv, dtype=np.float32)
    mask = np.asarray(mask)

    with_bias = bool(bq.any() or bk.any() or bv.any())
    key = ("nc", with_bias)
    if key not in _CACHE:
        _CACHE[key] = _build(with_bias=with_bias)
    nc = _CACHE[key]

    in_maps = [_prep_core_inputs(c, x, Wq, bq, Wk, bk, Wv, bv, mask,
                                 with_bias)
               for c in range(NCORES)]
    res = run_bass_kernel_spmd(nc, in_maps, core_ids=list(range(NCORES)),
                               trace=_trace)
    if _trace:
        _CACHE["last_result"] = res

    full = np.empty((B, S, D), dtype=np.float32)
    for c in range(NCORES):
        b, hg = c // 2, c % 2
        full[b, :, hg * DL:(hg + 1) * DL] = res.results[c]["out"].T
    return full


# revision 15
# speedup vs baseline: 1.3854x; 1.1199x over previous
"""Self-contained Trainium2 Bass kernel for a 12-head attention layer.

Problem: x[4,2048,768] -> attention(QKV projections, softmax, context),
NUM_HEADS=12, SIZE_PER_HEAD=64, additive mask from mask[4,2048].

Sharding over 8 NeuronCores: core c handles batch b=c//2 and head-group
hg=c%2 (6 heads, 384 feature columns).  Everything is local per core:
no collectives.

v3 design (ACT-bound): the per-core exp work (6 heads x 2048^2 = 25.2M
elements at 1 elem/lane/cycle @1.2GHz + 352c/inst) floors the Scalar
engine at ~220us with N=1024 ACTs, so the TensorEngine stream is
organized to fit just under that roof:

  per head-pair p, f-chunk g (512 wide), t-tile ti:
    scores  S^T[t,f]: head A -> psS[:,0:512], head B -> psS[:,512:1024]
            (one [128,1024] psum tile, two banks, double-buffered)
    exp     ONE ACT N=1024 over both heads' chunks:
            Exp(psS + adder[ti]) -> bf16 [128,1024]  (mask = ACT bias)
    ctx     per head: [65,512] psum accumulated over ti; the V tiles
            carry a 65th ones-column so row 64 is the softmax
            denominator (no separate denominator matmuls)
  normalization fully off the PE: DVE drain, DMA gather of denom rows,
  DVE reciprocal, gpsimd partition_broadcast, DVE multiply, DMA out.

Per-cycle budget: ACT 1147ns vs PE 2x scores + 2x ctx ~ 950ns + shared
projection work ~ 290ns -> both engines ~saturated, ACT binding.

Output per core: ctx^T [384,2048] f32; host transposes/concats.
"""

import numpy as np
import ml_dtypes

B, S, D = 4, 2048, 768
H, DH = 12, 64
HL = 6          # heads per core
DL = HL * DH    # 384 feature columns per core
NCORES = 8
P = 128
KO = 6          # full k-subtiles of the 768 contraction
NT = S // P     # 16 T-tiles
NG = 4          # f-chunks of 512 per head

_CACHE = {}


def _build(with_bias=False, ncores=NCORES):
    import concourse.mybir as mybir
    import concourse.tile as tile
    from concourse import bacc

    dt = mybir.dt
    Exp = mybir.ActivationFunctionType.Exp
    Alu = mybir.AluOpType

    nc = bacc.Bacc("TRN2", target_bir_lowering=False, debug=False,
                   num_devices=ncores)

    DE = D + 1 if with_bias else D
    WVC = HL * (DH + 1) if with_bias else DL   # 390 vs 384
    xT = nc.dram_tensor("xT", [DE, S], dt.bfloat16, kind="ExternalInput")
    wq = nc.dram_tensor("wq", [DE, DL], dt.bfloat16, kind="ExternalInput")
    wk = nc.dram_tensor("wk", [DE, DL], dt.bfloat16, kind="ExternalInput")
    wv = nc.dram_tensor("wv", [DE, WVC], dt.bfloat16, kind="ExternalInput")
    adder = nc.dram_tensor("adder", [P, NT], dt.float32, kind="ExternalInput")
    out = nc.dram_tensor("out", [DL, S], dt.float32, kind="ExternalOutput")

    KE = KO + 1 if with_bias else KO

    with tile.TileContext(nc) as tc:
        with (
            tc.tile_pool(name="persist", bufs=1) as sb,
            tc.tile_pool(name="work", bufs=3) as work,
            tc.tile_pool(name="fin", bufs=2) as fin,
            tc.tile_pool(name="ps_s", bufs=2, space="PSUM") as ps_s,
            tc.tile_pool(name="ps_c", bufs=1, space="PSUM") as ps_c,
        ):
            # ---- input DMA ----
            xTs = sb.tile([P, KE, S], dt.bfloat16, tag="xTs")
            for ko in range(KO):
                nc.sync.dma_start(
                    xTs[:, ko, :], xT.ap()[ko * P:(ko + 1) * P, :])
            if with_bias:
                nc.sync.dma_start(xTs[0:1, KO, :], xT.ap()[D:D + 1, :])

            wqs = sb.tile([P, KE, DL], dt.bfloat16, tag="wqs")
            wks = sb.tile([P, KE, DL], dt.bfloat16, tag="wks")
            wvs = sb.tile([P, KE, WVC], dt.bfloat16, tag="wvs")
            adder_sb = sb.tile([P, NT], dt.float32, tag="adder")

            def dma_w(w_dram, w_sb, cols):
                nc.sync.dma_start(
                    w_sb[:, 0:KO, 0:cols],
                    w_dram.ap()[0:D, :].rearrange("(ko p) m -> p ko m", p=P))
                if with_bias:
                    nc.sync.dma_start(w_sb[0:1, KO, 0:cols],
                                      w_dram.ap()[D:D + 1, :])

            dma_w(wq, wqs, DL)
            dma_w(wk, wks, DL)
            dma_w(wv, wvs, WVC)
            nc.sync.dma_start(adder_sb[:], adder.ap())

            # persistent projection outputs
            qt = sb.tile([P, 3, S], dt.bfloat16, tag="qt")   # Q^T/8 (+bias)
            kt = sb.tile([P, 3, S], dt.bfloat16, tag="kt")   # K^T (+bias)
            # V' token-major, 65-col head blocks (65th col = ones -> denom)
            vp = sb.tile([P, NT, HL, DH + 1], dt.bfloat16, tag="vp")
            if not with_bias:
                nc.gpsimd.memset(vp[:, :, :, DH:DH + 1], 1.0)

            # ---- projections ----
            def proj_qk(w_sb, dst, m, ns=(0, 1, 2, 3)):
                for n in ns:
                    pt = ps_c.tile([P, 512], dt.float32, tag="proj",
                                   name="pt")
                    for k in range(KE):
                        lhsT = (w_sb[:, k, m * P:(m + 1) * P] if k < KO
                                else w_sb[0:1, k, m * P:(m + 1) * P])
                        rhs = (xTs[:, k, n * 512:(n + 1) * 512] if k < KO
                               else xTs[0:1, k, n * 512:(n + 1) * 512])
                        nc.tensor.matmul(pt[:], lhsT, rhs,
                                         start=(k == 0), stop=(k == KE - 1))
                    nc.vector.tensor_copy(dst[:, m, n * 512:(n + 1) * 512],
                                          pt[:])

            def proj_v(mt):
                pt = ps_c.tile([P, 512], dt.float32, tag="proj", name="pt")
                for k in range(KE):
                    lhsT = (xTs[:, k, mt * P:(mt + 1) * P] if k < KO
                            else xTs[0:1, k, mt * P:(mt + 1) * P])
                    rhs = wvs[:, k, 0:WVC] if k < KO else wvs[0:1, k, 0:WVC]
                    nc.tensor.matmul(pt[:, :WVC], lhsT, rhs,
                                     start=(k == 0), stop=(k == KE - 1))
                if with_bias:
                    nc.vector.tensor_copy(
                        vp[:, mt, :, :],
                        pt[:, :WVC].rearrange("p (h c) -> p h c", h=HL))
                else:
                    nc.vector.tensor_copy(
                        vp[:, mt, :, 0:DH],
                        pt[:, :DL].rearrange("p (h c) -> p h c", h=HL))

            def epilogue(p, g, ctx_ps):
                # ctx_ps: [hip] -> [DH+1, 512] psum; row DH = denominator.
                # Drain psum fast (DVE), then the off-PE normalize chain.
                # DVE lanes are partition-locked, so each head stages at
                # base partition 0 and the output DMA moves the rows.
                csts = []
                for hip in range(2):
                    cst = fin.tile([DH + 1, 512], dt.float32,
                                   tag=f"cst{hip}", name="cst", bufs=2)
                    nc.vector.tensor_copy(cst[:], ctx_ps[hip][:])
                    csts.append(cst)
                dcol = fin.tile([P, 8], dt.float32, tag="dcol", name="dcol")
                for hip in range(2):
                    nc.sync.dma_start(dcol[:, hip * 4:(hip + 1) * 4],
                                      csts[hip][DH:DH + 1, :])
                rc = fin.tile([P, 8], dt.float32, tag="rc", name="rc")
                nc.vector.reciprocal(rc[:], dcol[:])
                for hip in range(2):
                    rrow = fin.tile([1, 512], dt.float32, tag="rrow",
                                    name="rrow")
                    nc.sync.dma_start(rrow[:],
                                      rc[:, hip * 4:(hip + 1) * 4])
                    bb = fin.tile([DH, 512], dt.float32, tag="bb",
                                  name="bb")
                    nc.gpsimd.partition_broadcast(bb[:], rrow[:])
                    otc = fin.tile([DH, 512], dt.float32, tag="otc",
                                   bufs=3, name="otc")
                    nc.vector.tensor_tensor(otc[:], csts[hip][0:DH, :],
                                            bb[:], Alu.mult)
                    nc.sync.dma_start(
                        out.ap()[p * P + hip * DH:p * P + (hip + 1) * DH,
                                 g * 512:(g + 1) * 512],
                        otc[:])

            def attn_all(bg):
                # One flattened software pipeline over all 192
                # (pair, g, ti) steps: scores+exp for step j, ctx for step
                # j-LAG.  The constant lag keeps the PE FIFO from ever
                # blocking on the ACT (ctx deps are LAG exp-cycles old)
                # and removes every segment/pair boundary bubble.  One
                # background-projection instruction (thunk) interleaves
                # per step into the PE slack under the ACT roof.
                LAG = 4
                ctx_tiles = {}
                exp_tiles = {}

                def seg_of(step):
                    p, r = divmod(step, 64)
                    return p, r // 16, r % 16

                for step in range(192 + LAG):
                    if step < 192:
                        p, g, ti = seg_of(step)
                        psS = ps_s.tile([P, 1024], dt.float32, tag="s",
                                        name="psS")
                        nc.tensor.matmul(
                            psS[:, 0:512],
                            kt[0:DH, p, ti * P:(ti + 1) * P],
                            qt[0:DH, p, g * 512:(g + 1) * 512],
                            start=True, stop=True)
                        nc.tensor.matmul(
                            psS[:, 512:1024],
                            kt[DH:P, p, ti * P:(ti + 1) * P],
                            qt[DH:P, p, g * 512:(g + 1) * 512],
                            start=True, stop=True)
                        et = work.tile([P, 1024], dt.bfloat16, tag="exp",
                                       name="et", bufs=LAG + 2)
                        nc.scalar.activation(et[:], psS[:], Exp,
                                             bias=adder_sb[:, ti:ti + 1],
                                             scale=1.0)
                        exp_tiles[step] = et
                    j = step - LAG
                    if j >= 0:
                        p, g, ti = seg_of(j)
                        seg = 4 * p + g
                        if ti == 0:
                            ctx_tiles[seg] = [
                                ps_c.tile([DH + 1, 512], dt.float32,
                                          tag=f"c{hip}", name="ctx_ps")
                                for hip in range(2)
                            ]
                        et = exp_tiles.pop(j)
                        for hip in range(2):
                            nc.tensor.matmul(
                                ctx_tiles[seg][hip][:],
                                vp[:, ti, 2 * p + hip, :],
                                et[:, hip * 512:(hip + 1) * 512],
                                start=(ti == 0), stop=(ti == NT - 1))
                        if ti == NT - 1:
                            epilogue(p, g, ctx_tiles.pop(seg))
                    if bg:
                        bg.pop(0)()

            # PE warm-up: garbage matmuls with no input deps run during the
            # initial DMA wait, releasing the HAM clock throttle.
            warm = sb.tile([P, 512], dt.bfloat16, tag="warm")
            nc.gpsimd.memset(warm[:], 0.0)
            wexp = sb.tile([P, 1], dt.bfloat16, tag="wexp")
            nc.scalar.activation(wexp[:], warm[:, 0:1], Exp)
            wpt = ps_s.tile([P, 1024], dt.float32, tag="s", name="wpt")
            for wi in range(20):
                nc.tensor.matmul(wpt[:, 0:512], warm[:, 0:P], warm[:],
                                 start=(wi == 0), stop=(wi == 19))
            wpt2 = ps_s.tile([P, 1024], dt.float32, tag="s", name="wpt2")
            for wi in range(28):
                nc.tensor.matmul(wpt2[:, 0:256], warm[:, 0:P],
                                 warm[:, 0:256],
                                 start=(wi == 0), stop=(wi == 27))

            def proj_thunks_qk(w_sb, dst, m, n):
                # one-instruction-per-thunk version of proj_qk(m, (n,))
                state = {}

                def mk(k):
                    def t():
                        if k == 0:
                            state["pt"] = ps_c.tile([P, 512], dt.float32,
                                                    tag="proj", name="pt")
                        lhsT = (w_sb[:, k, m * P:(m + 1) * P] if k < KO
                                else w_sb[0:1, k, m * P:(m + 1) * P])
                        rhs = (xTs[:, k, n * 512:(n + 1) * 512] if k < KO
                               else xTs[0:1, k, n * 512:(n + 1) * 512])
                        nc.tensor.matmul(state["pt"][:], lhsT, rhs,
                                         start=(k == 0), stop=(k == KE - 1))
                    return t

                def cp():
                    nc.vector.tensor_copy(
                        dst[:, m, n * 512:(n + 1) * 512], state["pt"][:])

                return [mk(k) for k in range(KE)] + [cp]

            # prefix: only what the first attention steps need up front
            proj_qk(wqs, qt, 0, ns=(0,))
            proj_qk(wks, kt, 0)
            for mt in range(NT):
                proj_v(mt)
            bg = []
            for n in (1, 2, 3):
                bg += proj_thunks_qk(wqs, qt, 0, n)
            for n in range(4):
                bg += proj_thunks_qk(wks, kt, 1, n)
            for n in range(4):
                bg += proj_thunks_qk(wqs, qt, 1, n)
            for n in range(4):
                bg += proj_thunks_qk(wks, kt, 2, n)
            for n in range(4):
                bg += proj_thunks_qk(wqs, qt, 2, n)
            bg += [lambda: None] * (200 - len(bg))
            attn_all(bg)

    nc.compile()
    return nc


def _prep_core_inputs(c, x, Wq, bq, Wk, bk, Wv, bv, mask, with_bias):
    bf16 = ml_dtypes.bfloat16
    b, hg = c // 2, c % 2
    cols = slice(hg * DL, (hg + 1) * DL)
    DE = D + 1 if with_bias else D

    xT_aug = np.empty((DE, S), dtype=bf16)
    xT_aug[:D] = x[b].T.astype(bf16)
    if with_bias:
        xT_aug[D] = np.float32(1.0)

    wq_aug = np.empty((DE, DL), dtype=bf16)
    wq_aug[:D] = (Wq[:, cols] / 8.0).astype(bf16)
    wk_aug = np.empty((DE, DL), dtype=bf16)
    wk_aug[:D] = Wk[:, cols].astype(bf16)
    if with_bias:
        wq_aug[D] = (bq[cols] / 8.0).astype(bf16)
        wk_aug[D] = bk[cols].astype(bf16)
        wv_aug = np.zeros((DE, HL * (DH + 1)), dtype=bf16)
        wv_loc = Wv[:, cols].astype(np.float32)
        bv_loc = bv[cols].astype(np.float32)
        for j in range(HL):
            wv_aug[:D, j * (DH + 1):j * (DH + 1) + DH] = \
                wv_loc[:, j * DH:(j + 1) * DH].astype(bf16)
            wv_aug[D, j * (DH + 1):j * (DH + 1) + DH] = \
                bv_loc[j * DH:(j + 1) * DH].astype(bf16)
            wv_aug[D, j * (DH + 1) + DH] = np.float32(1.0)
    else:
        wv_aug = np.empty((DE, DL), dtype=bf16)
        wv_aug[:D] = Wv[:, cols].astype(bf16)

    add = ((mask[b].astype(np.float32) - 1.0) * 10000.0)
    adder_t = add.reshape(NT, P).T.copy()   # [128,16]: [p, ti]

    return {"xT": xT_aug, "wq": wq_aug, "wk": wk_aug, "wv": wv_aug,
            "adder": np.ascontiguousarray(adder_t, dtype=np.float32)}


def kernel(x, Wq, bq, Wk, bk, Wv, bv, mask, _trace=False):
    from concourse.bass_utils import run_bass_kernel_spmd

    x = np.asarray(x, dtype=np.float32)
    Wq = np.asarray(Wq, dtype=np.float32)
    bq = np.asarray(bq, dtype=np.float32)
    Wk = np.asarray(Wk, dtype=np.float32)
    bk = np.asarray(bk, dtype=np.float32)
    Wv = np.asarray(Wv, dtype=np.float32)
    bv = np.asarray(bv, dtype=np.float32)
    mask = np.asarray(mask)

    with_bias = bool(bq.any() or bk.any() or bv.any())
    key = ("nc", with_bias)
    if key not in _CACHE:
        _CACHE[key] = _build(with_bias=with_bias)
    nc = _CACHE[key]

    in_maps = [_prep_core_inputs(c, x, Wq, bq, Wk, bk, Wv, bv, mask,
                                 with_bias)
               for c in range(NCORES)]
    res = run_bass_kernel_spmd(nc, in_maps, core_ids=list(range(NCORES)),
                               trace=_trace)
    if _trace:
        _CACHE["last_result"] = res

    full = np.empty((B, S, D), dtype=np.float32)
    for c in range(NCORES):
        b, hg = c // 2, c % 2
        full[b, :, hg * DL:(hg + 1) * DL] = res.results[c]["out"].T
    return full


# revision 22
# speedup vs baseline: 1.4352x; 1.0360x over previous
"""Self-contained Trainium2 Bass kernel for a 12-head attention layer.

Problem: x[4,2048,768] -> attention(QKV projections, softmax, context),
NUM_HEADS=12, SIZE_PER_HEAD=64, additive mask from mask[4,2048].

Sharding over 8 NeuronCores: core c handles batch b=c//2 and head-group
hg=c%2 (6 heads, 384 feature columns).  Everything is local per core:
no collectives.

v3 design (ACT-bound): the per-core exp work (6 heads x 2048^2 = 25.2M
elements at 1 elem/lane/cycle @1.2GHz + 352c/inst) floors the Scalar
engine at ~220us with N=1024 ACTs, so the TensorEngine stream is
organized to fit just under that roof:

  per head-pair p, f-chunk g (512 wide), t-tile ti:
    scores  S^T[t,f]: head A -> psS[:,0:512], head B -> psS[:,512:1024]
            (one [128,1024] psum tile, two banks, double-buffered)
    exp     ONE ACT N=1024 over both heads' chunks:
            Exp(psS + adder[ti]) -> bf16 [128,1024]  (mask = ACT bias)
    ctx     per head: [65,512] psum accumulated over ti; the V tiles
            carry a 65th ones-column so row 64 is the softmax
            denominator (no separate denominator matmuls)
  normalization fully off the PE: DVE drain, DMA gather of denom rows,
  DVE reciprocal, gpsimd partition_broadcast, DVE multiply, DMA out.

Per-cycle budget: ACT 1147ns vs PE 2x scores + 2x ctx ~ 950ns + shared
projection work ~ 290ns -> both engines ~saturated, ACT binding.

Output per core: ctx^T [384,2048] f32; host transposes/concats.
"""

import numpy as np
import ml_dtypes

B, S, D = 4, 2048, 768
H, DH = 12, 64
HL = 6          # heads per core
DL = HL * DH    # 384 feature columns per core
NCORES = 8
P = 128
KO = 6          # full k-subtiles of the 768 contraction
NT = S // P     # 16 T-tiles
NG = 4          # f-chunks of 512 per head

_CACHE = {}


def _build(with_bias=False, ncores=NCORES):
    import concourse.mybir as mybir
    import concourse.tile as tile
    from concourse import bacc

    dt = mybir.dt
    Exp = mybir.ActivationFunctionType.Exp
    Alu = mybir.AluOpType

    nc = bacc.Bacc("TRN2", target_bir_lowering=False, debug=False,
                   num_devices=ncores)

    DE = D + 1 if with_bias else D
    WVC = HL * (DH + 1) if with_bias else DL   # 390 vs 384
    xT = nc.dram_tensor("xT", [DE, S], dt.bfloat16, kind="ExternalInput")
    wq = nc.dram_tensor("wq", [DE, DL], dt.bfloat16, kind="ExternalInput")
    wk = nc.dram_tensor("wk", [DE, DL], dt.bfloat16, kind="ExternalInput")
    wv = nc.dram_tensor("wv", [DE, WVC], dt.bfloat16, kind="ExternalInput")
    adder = nc.dram_tensor("adder", [P, NT], dt.float32, kind="ExternalInput")
    out = nc.dram_tensor("out", [DL, S], dt.float32, kind="ExternalOutput")

    KE = KO + 1 if with_bias else KO

    with tile.TileContext(nc) as tc:
        with (
            tc.tile_pool(name="persist", bufs=1) as sb,
            tc.tile_pool(name="work", bufs=3) as work,
            tc.tile_pool(name="fin", bufs=2) as fin,
            tc.tile_pool(name="ps_s", bufs=2, space="PSUM") as ps_s,
            tc.tile_pool(name="ps_c", bufs=1, space="PSUM") as ps_c,
        ):
            # ---- input DMA ----
            xTs = sb.tile([P, KE, S], dt.bfloat16, tag="xTs")
            for ko in range(KO):
                nc.sync.dma_start(
                    xTs[:, ko, :], xT.ap()[ko * P:(ko + 1) * P, :])
            if with_bias:
                nc.sync.dma_start(xTs[0:1, KO, :], xT.ap()[D:D + 1, :])

            wqs = sb.tile([P, KE, DL], dt.bfloat16, tag="wqs")
            wks = sb.tile([P, KE, DL], dt.bfloat16, tag="wks")
            wvs = sb.tile([P, KE, WVC], dt.bfloat16, tag="wvs")
            adder_sb = sb.tile([P, NT], dt.float32, tag="adder")

            def dma_w(w_dram, w_sb, cols):
                nc.sync.dma_start(
                    w_sb[:, 0:KO, 0:cols],
                    w_dram.ap()[0:D, :].rearrange("(ko p) m -> p ko m", p=P))
                if with_bias:
                    nc.sync.dma_start(w_sb[0:1, KO, 0:cols],
                                      w_dram.ap()[D:D + 1, :])

            dma_w(wq, wqs, DL)
            dma_w(wk, wks, DL)
            dma_w(wv, wvs, WVC)
            nc.sync.dma_start(adder_sb[:], adder.ap())

            # persistent projection outputs
            qt = sb.tile([P, 3, S], dt.bfloat16, tag="qt")   # Q^T/8 (+bias)
            kt = sb.tile([P, 3, S], dt.bfloat16, tag="kt")   # K^T (+bias)
            # V' token-major, 65-col head blocks (65th col = ones -> denom)
            vp = sb.tile([P, NT, HL, DH + 1], dt.bfloat16, tag="vp")
            if not with_bias:
                nc.gpsimd.memset(vp[:, :, :, DH:DH + 1], 1.0)

            # ---- projections ----
            def proj_qk(w_sb, dst, m, ns=(0, 1, 2, 3)):
                for n in ns:
                    pt = ps_c.tile([P, 512], dt.float32, tag="proj",
                                   name="pt", bufs=2)
                    for k in range(KE):
                        lhsT = (w_sb[:, k, m * P:(m + 1) * P] if k < KO
                                else w_sb[0:1, k, m * P:(m + 1) * P])
                        rhs = (xTs[:, k, n * 512:(n + 1) * 512] if k < KO
                               else xTs[0:1, k, n * 512:(n + 1) * 512])
                        nc.tensor.matmul(pt[:], lhsT, rhs,
                                         start=(k == 0), stop=(k == KE - 1))
                    nc.vector.tensor_copy(dst[:, m, n * 512:(n + 1) * 512],
                                          pt[:])

            def proj_v(mt):
                pt = ps_c.tile([P, 512], dt.float32, tag="proj", name="pt",
                               bufs=2)
                for k in range(KE):
                    lhsT = (xTs[:, k, mt * P:(mt + 1) * P] if k < KO
                            else xTs[0:1, k, mt * P:(mt + 1) * P])
                    rhs = wvs[:, k, 0:WVC] if k < KO else wvs[0:1, k, 0:WVC]
                    nc.tensor.matmul(pt[:, :WVC], lhsT, rhs,
                                     start=(k == 0), stop=(k == KE - 1))
                if with_bias:
                    nc.vector.tensor_copy(
                        vp[:, mt, :, :],
                        pt[:, :WVC].rearrange("p (h c) -> p h c", h=HL))
                else:
                    nc.vector.tensor_copy(
                        vp[:, mt, :, 0:DH],
                        pt[:, :DL].rearrange("p (h c) -> p h c", h=HL))

            def epilogue(p, g, ctx_ps, fine=False):
                # ctx_ps: [hip] -> [DH+1, 512] psum; row DH = denominator.
                # Drain psum fast (DVE), then the off-PE normalize chain.
                # DVE lanes are partition-locked, so each head stages at
                # base partition 0 and the output DMA moves the rows.
                # fine=True splits into 256-wide chains so the kernel tail
                # pipelines across DVE/DMA/GpSimd.
                csts = []
                for hip in range(2):
                    cst = fin.tile([DH + 1, 512], dt.float32,
                                   tag=f"cst{hip}", name="cst", bufs=2)
                    nc.vector.tensor_copy(cst[:], ctx_ps[hip][:])
                    csts.append(cst)
                nch = 2 if fine else 1
                cw = 512 // nch
                for ch in range(nch):
                    dcol = fin.tile([P, 8 // nch], dt.float32, tag=f"dcol{nch}",
                                    name="dcol", bufs=2 * nch)
                    for hip in range(2):
                        nc.sync.dma_start(
                            dcol[:, hip * 4 // nch:(hip + 1) * 4 // nch],
                            csts[hip][DH:DH + 1, ch * cw:(ch + 1) * cw])
                    rc = fin.tile([P, 8 // nch], dt.float32, tag=f"rc{nch}",
                                  name="rc", bufs=2 * nch)
                    nc.vector.reciprocal(rc[:], dcol[:])
                    for hip in range(2):
                        rrow = fin.tile([1, cw], dt.float32, tag=f"rrow{nch}",
                                        name="rrow", bufs=2 * nch)
                        nc.sync.dma_start(
                            rrow[:],
                            rc[:, hip * 4 // nch:(hip + 1) * 4 // nch])
                        bb = fin.tile([DH, cw], dt.float32, tag=f"bb{nch}",
                                      name="bb", bufs=2 * nch)
                        nc.gpsimd.partition_broadcast(bb[:], rrow[:])
                        otc = fin.tile([DH, cw], dt.float32, tag=f"otc{nch}",
                                       bufs=3 * nch, name="otc")
                        nc.vector.tensor_tensor(
                            otc[:],
                            csts[hip][0:DH, ch * cw:(ch + 1) * cw],
                            bb[:], Alu.mult)
                        nc.sync.dma_start(
                            out.ap()[p * P + hip * DH:
                                     p * P + (hip + 1) * DH,
                                     g * 512 + ch * cw:
                                     g * 512 + (ch + 1) * cw],
                            otc[:])

            def attn_all(bg, pre_step):
                # One flattened software pipeline over all 192
                # (pair, g, ti) steps: scores+exp for step j, ctx for step
                # j-16.  The segment-sized lag means ctx deps are a full
                # exp-backlog old (the PE FIFO never waits on the ACT),
                # segment s's ctx drains exactly during segment s+1's
                # scores (ctx psum tiles stay single-buffered), and every
                # segment/pair boundary bubble disappears.  pre_step maps
                # step -> emission block run before that step (V-proj
                # blocks that hide under the exp backlog); bg interleaves
                # one background-projection instruction per step.
                LAG = NT
                ctx_tiles = {}
                exp_tiles = {}

                def seg_of(step):
                    p, r = divmod(step, 64)
                    return p, r // 16, r % 16

                for step in range(192 + LAG):
                    if step in pre_step:
                        pre_step[step]()
                    if step < 192:
                        p, g, ti = seg_of(step)
                        psS = ps_s.tile([P, 1024], dt.float32, tag="s",
                                        name="psS")
                        nc.tensor.matmul(
                            psS[:, 0:512],
                            kt[0:DH, p, ti * P:(ti + 1) * P],
                            qt[0:DH, p, g * 512:(g + 1) * 512],
                            start=True, stop=True)
                        nc.tensor.matmul(
                            psS[:, 512:1024],
                            kt[DH:P, p, ti * P:(ti + 1) * P],
                            qt[DH:P, p, g * 512:(g + 1) * 512],
                            start=True, stop=True)
                        et = work.tile([P, 1024], dt.bfloat16, tag="exp",
                                       name="et", bufs=LAG + 2)
                        nc.scalar.activation(et[:], psS[:], Exp,
                                             bias=adder_sb[:, ti:ti + 1],
                                             scale=1.0)
                        exp_tiles[step] = et
                    j = step - LAG
                    if j >= 0:
                        p, g, ti = seg_of(j)
                        seg = 4 * p + g
                        if ti == 0:
                            ctx_tiles[seg] = [
                                ps_c.tile([DH + 1, 512], dt.float32,
                                          tag=f"c{hip}", name="ctx_ps")
                                for hip in range(2)
                            ]
                        et = exp_tiles.pop(j)
                        for hip in range(2):
                            nc.tensor.matmul(
                                ctx_tiles[seg][hip][:],
                                vp[:, ti, 2 * p + hip, :],
                                et[:, hip * 512:(hip + 1) * 512],
                                start=(ti == 0), stop=(ti == NT - 1))
                        if ti == NT - 1:
                            epilogue(p, g, ctx_tiles.pop(seg),
                                     fine=(seg == 11))
                    if bg:
                        bg.pop(0)()

            # PE warm-up: garbage matmuls with no input deps run during the
            # initial DMA wait, releasing the HAM clock throttle.
            warm = sb.tile([P, 512], dt.bfloat16, tag="warm")
            nc.gpsimd.memset(warm[:], 0.0)
            wexp = sb.tile([P, 1], dt.bfloat16, tag="wexp")
            nc.scalar.activation(wexp[:], warm[:, 0:1], Exp)
            wpt = ps_s.tile([P, 1024], dt.float32, tag="s", name="wpt")
            for wi in range(20):
                nc.tensor.matmul(wpt[:, 0:512], warm[:, 0:P], warm[:],
                                 start=(wi == 0), stop=(wi == 19))
            wpt2 = ps_s.tile([P, 1024], dt.float32, tag="s", name="wpt2")
            for wi in range(28):
                nc.tensor.matmul(wpt2[:, 0:256], warm[:, 0:P],
                                 warm[:, 0:256],
                                 start=(wi == 0), stop=(wi == 27))

            def proj_thunks_qk(w_sb, dst, m, n):
                # one-instruction-per-thunk version of proj_qk(m, (n,))
                state = {}

                def mk(k):
                    def t():
                        if k == 0:
                            state["pt"] = ps_c.tile([P, 512], dt.float32,
                                                    tag="proj", name="pt",
                                                    bufs=2)
                        lhsT = (w_sb[:, k, m * P:(m + 1) * P] if k < KO
                                else w_sb[0:1, k, m * P:(m + 1) * P])
                        rhs = (xTs[:, k, n * 512:(n + 1) * 512] if k < KO
                               else xTs[0:1, k, n * 512:(n + 1) * 512])
                        nc.tensor.matmul(state["pt"][:], lhsT, rhs,
                                         start=(k == 0), stop=(k == KE - 1))
                    return t

                def cp():
                    nc.vector.tensor_copy(
                        dst[:, m, n * 512:(n + 1) * 512], state["pt"][:])

                return [mk(k) for k in range(KE)] + [cp]

            # prefix: only what g0's scores need up front; the V
            # projections run AFTER g0's 16 scores, hidden under the ACT's
            # exp backlog (pre_step blocks at steps 16 and 24).
            proj_qk(wqs, qt, 0, ns=(0,))
            proj_qk(wks, kt, 0)

            def v_block(lo, hi):
                def f():
                    for mt in range(lo, hi):
                        proj_v(mt)
                return f

            bg = []
            for n in (1, 2, 3):
                bg += proj_thunks_qk(wqs, qt, 0, n)
            for n in range(4):
                bg += proj_thunks_qk(wks, kt, 1, n)
            for n in range(4):
                bg += proj_thunks_qk(wqs, qt, 1, n)
            for n in range(4):
                bg += proj_thunks_qk(wks, kt, 2, n)
            for n in range(4):
                bg += proj_thunks_qk(wqs, qt, 2, n)
            bg += [lambda: None] * (220 - len(bg))
            attn_all(bg, {16: v_block(0, 8), 24: v_block(8, NT)})

    nc.compile()
    return nc


def _prep_core_inputs(c, x, Wq, bq, Wk, bk, Wv, bv, mask, with_bias):
    bf16 = ml_dtypes.bfloat16
    b, hg = c // 2, c % 2
    cols = slice(hg * DL, (hg + 1) * DL)
    DE = D + 1 if with_bias else D

    xT_aug = np.empty((DE, S), dtype=bf16)
    xT_aug[:D] = x[b].T.astype(bf16)
    if with_bias:
        xT_aug[D] = np.float32(1.0)

    wq_aug = np.empty((DE, DL), dtype=bf16)
    wq_aug[:D] = (Wq[:, cols] / 8.0).astype(bf16)
    wk_aug = np.empty((DE, DL), dtype=bf16)
    wk_aug[:D] = Wk[:, cols].astype(bf16)
    if with_bias:
        wq_aug[D] = (bq[cols] / 8.0).astype(bf16)
        wk_aug[D] = bk[cols].astype(bf16)
        wv_aug = np.zeros((DE, HL * (DH + 1)), dtype=bf16)
        wv_loc = Wv[:, cols].astype(np.float32)
        bv_loc = bv[cols].astype(np.float32)
        for j in range(HL):
            wv_aug[:D, j * (DH + 1):j * (DH + 1) + DH] = \
                wv_loc[:, j * DH:(j + 1) * DH].astype(bf16)
            wv_aug[D, j * (DH + 1):j * (DH + 1) + DH] = \
                bv_loc[j * DH:(j + 1) * DH].astype(bf16)
            wv_aug[D, j * (DH + 1) + DH] = np.float32(1.0)
    else:
        wv_aug = np.empty((DE, DL), dtype=bf16)
        wv_aug[:D] = Wv[:, cols].astype(bf16)

    add = ((mask[b].astype(np.float32) - 1.0) * 10000.0)
    adder_t = add.reshape(NT, P).T.copy()   # [128,16]: [p, ti]

    return {"xT": xT_aug, "wq": wq_aug, "wk": wk_aug, "wv": wv_aug,
            "adder": np.ascontiguousarray(adder_t, dtype=np.float32)}


def kernel(x, Wq, bq, Wk, bk, Wv, bv, mask, _trace=False):
    from concourse.bass_utils import run_bass_kernel_spmd

    x = np.asarray(x, dtype=np.float32)
    Wq = np.asarray(Wq, dtype=np.float32)
    bq = np.asarray(bq, dtype=np.float32)
    Wk = np.asarray(Wk, dtype=np.float32)
    bk = np.asarray(bk, dtype=np.float32)
    Wv = np.asarray(Wv, dtype=np.float32)
    bv = np.asarray(bv, dtype=np.float32)
    mask = np.asarray(mask)

    with_bias = bool(bq.any() or bk.any() or bv.any())
    key = ("nc", with_bias)
    if key not in _CACHE:
        _CACHE[key] = _build(with_bias=with_bias)
    nc = _CACHE[key]

    in_maps = [_prep_core_inputs(c, x, Wq, bq, Wk, bk, Wv, bv, mask,
                                 with_bias)
               for c in range(NCORES)]
    res = run_bass_kernel_spmd(nc, in_maps, core_ids=list(range(NCORES)),
                               trace=_trace)
    if _trace:
        _CACHE["last_result"] = res

    full = np.empty((B, S, D), dtype=np.float32)
    for c in range(NCORES):
        b, hg = c // 2, c % 2
        full[b, :, hg * DL:(hg + 1) * DL] = res.results[c]["out"].T
    return full
